# revision 1
# baseline (speedup 1.0000x reference)
"""DeepseekV2 MLA decode attention on 8 Trainium2 NeuronCores.

Strategy (single SPMD launch, identical program on all cores; all per-core
variation comes from in_maps contents and collective semantics):

  - Attention is batch-sharded: core k owns sequences 4k..4k+4, whose KV
    cache slices are fed to it via in_maps in TWO host-prepared layouts:
    natural [s, c] (context matmul, contracts s) and transposed [c, s]
    (score matmul, contracts c). The PE contracts along partitions, so the
    two matmuls need different partition assignments of the same data;
    host-side dual layout avoids all on-chip cache transposition.
  - Scores are computed transposed (PSUM [128 s, 16 h]) so the exp (ACT)
    writes e_T directly in the layout the context matmul consumes as its
    stationary operand.
  - w_qkv_a is K-sharded (hidden dim / 8); the row-major partial qkv
    activations are ReduceScattered, which both sums the partials and
    hands each core exactly its 4 sequences (rank-dependent slicing via
    collective semantics).
  - w_o is column-sharded; ctx_v rows are AllGathered and each core
    produces a 640-column slice of the output, concatenated on host.
  - q_a_norm_w is folded into w_q_b on the host (rmsnorm scale is diag).
  - The current-token cache update (rmsnorm latent / roped k_pe written
    at slot S-1) is applied on the host while building the cache layouts.
"""

import sys

sys.path.insert(0, "/opt/trn_rl_repo")

import numpy as np

import concourse.bacc as bacc
import concourse.mybir as mybir
import concourse.tile as tile
from concourse import bass_utils
from concourse.masks import make_identity

F32 = mybir.dt.float32
ADD = mybir.AluOpType.add
MULT = mybir.AluOpType.mult
BYPASS = mybir.AluOpType.bypass
EXP = mybir.ActivationFunctionType.Exp
SQRT = mybir.ActivationFunctionType.Sqrt
AXIS_X = mybir.AxisListType.X

B, HID, H = 32, 5120, 16
DN, DR, DV = 128, 64, 128
QL, KL = 1536, 512
BASE = 10000.0
EPS = 1e-6
SCALE = float((DN + DR) ** -0.5)

N_CORES = 8
BP = B // N_CORES      # sequences per core
NKT = QL // 128        # 12
TP = True              # collective-based weight sharding

_CACHE = {}


# ----------------------------- host math ---------------------------------


def _rmsnorm_np(x, w):
    ms = np.mean(x * x, axis=-1, keepdims=True, dtype=np.float32)
    return (x * (1.0 / np.sqrt(ms + EPS)) * w).astype(np.float32)


def _rope_np(x, pos):
    d = x.shape[-1]
    inv = (1.0 / (BASE ** (np.arange(0, d, 2, dtype=np.float32) / d))).astype(
        np.float32
    )
    fr = pos.astype(np.float32)[:, None] * inv
    cos, sin = np.cos(fr).astype(np.float32), np.sin(fr).astype(np.float32)
    out = np.empty_like(x)
    out[..., 0::2] = x[..., 0::2] * cos - x[..., 1::2] * sin
    out[..., 1::2] = x[..., 1::2] * cos + x[..., 0::2] * sin
    return out.astype(np.float32)


def _rope_RT(pos):
    """Per-batch transposed rotation matrices (lhsT for rope-as-matmul)."""
    inv = (1.0 / (BASE ** (np.arange(0, DR, 2, dtype=np.float32) / DR))).astype(
        np.float32
    )
    fr = pos.astype(np.float32)[:, None] * inv
    cos, sin = np.cos(fr).astype(np.float32), np.sin(fr).astype(np.float32)
    R = np.zeros((B, DR, DR), np.float32)
    j = np.arange(DR // 2)
    bi = np.arange(B)[:, None]
    R[bi, 2 * j, 2 * j] = cos
    R[bi, 2 * j, 2 * j + 1] = -sin
    R[bi, 2 * j + 1, 2 * j] = sin
    R[bi, 2 * j + 1, 2 * j + 1] = cos
    return np.ascontiguousarray(R.transpose(0, 2, 1))


# ----------------------------- device program ----------------------------


def _build(S, n_cores, tp, fake_coll=False, TRF=2):
    nc = bacc.Bacc("TRN2", target_bir_lowering=False, debug=False,
                   enable_asserts=False, num_devices=n_cores)
    ST = S // 512
    rg = [list(range(n_cores))]
    NB = B if tp else BP            # batch width of the qkv_a projection
    KTH = (HID // n_cores if tp else HID) // 128   # hidden k-tiles (5 / 40)
    HO = HID // n_cores if tp else HID             # output columns per core

    hT = nc.dram_tensor("hT", [128 * KTH, NB], F32, kind="ExternalInput")
    w_qa = nc.dram_tensor("w_qa", [128 * KTH, QL], F32, kind="ExternalInput")
    w_qb = nc.dram_tensor("w_qb", [QL, H * (DN + DR)], F32,
                          kind="ExternalInput")
    w_kc = nc.dram_tensor("w_kc", [H, DN, KL], F32, kind="ExternalInput")
    w_vc = nc.dram_tensor("w_vc", [H, KL, DV], F32, kind="ExternalInput")
    w_o = nc.dram_tensor("w_o", [H * DV, HO], F32, kind="ExternalInput")
    cache_nat = nc.dram_tensor("cache_nat", [BP, S, KL], F32,
                               kind="ExternalInput")
    cacheT_l = nc.dram_tensor("cacheT_l", [BP, KL, S], F32,
                              kind="ExternalInput")
    cacheT_r = nc.dram_tensor("cacheT_r", [BP, DR, S], F32,
                              kind="ExternalInput")
    ropeRT = nc.dram_tensor("ropeRT", [BP, DR, DR], F32, kind="ExternalInput")
    out = nc.dram_tensor("out", [NB if tp else BP, HO], F32,
                         kind="ExternalOutput")

    with tile.TileContext(nc) as tc:
        with (
            tc.tile_pool(name="const", bufs=1) as cp,
            tc.tile_pool(name="qsb", bufs=1) as qsb,
            tc.tile_pool(name="dram", bufs=1, space="DRAM") as dramp,
            tc.tile_pool(name="wstream", bufs=2) as wsp,
            tc.tile_pool(name="wo", bufs=1) as wop,
            tc.tile_pool(name="ctl", bufs=3) as ctlp,
            tc.tile_pool(name="ctr", bufs=1) as ctrp,
            tc.tile_pool(name="nat", bufs=4) as natp,
            tc.tile_pool(name="et", bufs=4) as etp,
            tc.tile_pool(name="small", bufs=1) as smp,
            tc.tile_pool(name="small2", bufs=2) as smp2,
        ):
            ones_col = cp.tile([128, 1], F32)
            nc.any.memset(ones_col, 1.0)
            eps_t = cp.tile([128, 1], F32)
            nc.any.memset(eps_t, EPS)
            ident = cp.tile([128, 128], F32)
            make_identity(nc, ident[:, :])
            rt_sb = cp.tile([DR, BP, DR], F32)
            nc.sync.dma_start(rt_sb[:, :, :],
                              ropeRT[:, :, :].rearrange("b k m -> k b m"))
            hT_sb = cp.tile([128, KTH, NB], F32)
            nc.sync.dma_start(hT_sb[:, :, :],
                              hT[:, :].rearrange("(t p) b -> p t b", p=128))

            # ================= q path =================
            with tc.tile_pool(name="psq", bufs=6, space="PSUM") as psq:

                def qps(name):
                    return psq.tile([128, 512], F32, tag="q", name=name)

                # ---- qkv_a projection: q_a rows [NB, 1536] ----
                qkv_rows = qsb.tile([NB, QL], F32)
                pss = [qps(f"qkv{j}") for j in range(3)]
                for kt in range(KTH):
                    wt = wsp.tile([128, 1536], F32, tag="wqa")
                    nc.sync.dma_start(wt[:, :],
                                      w_qa[kt * 128:(kt + 1) * 128, :])
                    for j in range(3):
                        nc.tensor.matmul(
                            pss[j][:NB, :], hT_sb[:, kt, :],
                            wt[:, j * 512:(j + 1) * 512],
                            start=(kt == 0), stop=(kt == KTH - 1))
                for j in range(3):
                    nc.any.tensor_copy(
                        qkv_rows[:, j * 512:(j + 1) * 512], pss[j][:NB, :])

                # ---- ReduceScatter partials -> my 4 sequences' q_a ----
                if tp:
                    rs_in = dramp.tile([B, QL], F32)
                    rs_out = dramp.tile([BP, QL], F32)
                    nc.sync.dma_start(rs_in[:, :], qkv_rows[:, :])
                    if fake_coll:
                        nc.sync.dma_start(rs_out[:, :], rs_in[0:BP, :])
                    else:
                        nc.gpsimd.collective_compute(
                            "ReduceScatter", ADD, replica_groups=rg,
                            ins=[rs_in.opt()], outs=[rs_out.opt()])
                    qa_mine = qsb.tile([BP, QL], F32)
                    nc.sync.dma_start(qa_mine[:, :], rs_out[:, :])
                else:
                    qa_mine = qkv_rows

                # ---- rmsnorm (rows) + transpose to [128, 12, 4] ----
                sq = smp.tile([BP, QL], F32, tag="sq")
                nc.vector.tensor_tensor(sq[:, :], qa_mine[:, :],
                                        qa_mine[:, :], MULT)
                ssum = smp.tile([BP, 1], F32, tag="ssum")
                nc.vector.reduce_sum(ssum[:, :], sq[:, :], AXIS_X)
                rms = smp.tile([BP, 1], F32, tag="rms")
                nc.scalar.activation(rms[:, :], ssum[:, :], SQRT,
                                     bias=eps_t[:BP, :1], scale=1.0 / QL)
                rinv = smp.tile([BP, 1], F32, tag="rinv")
                nc.vector.reciprocal(rinv[:, :], rms[:, :])
                qan = smp.tile([BP, QL], F32, tag="qan")
                nc.vector.tensor_scalar_mul(qan[:, :], qa_mine[:, :],
                                            rinv[:, :1])

                ps_t = qps("qanT")
                for t in range(NKT):
                    nc.tensor.transpose(ps_t[:, t * BP:(t + 1) * BP],
                                        qan[:BP, t * 128:(t + 1) * 128],
                                        ident[:BP, :BP])
                qanT = qsb.tile([128, NKT, BP], F32)
                nc.any.tensor_copy(qanT[:, :, :], ps_t[:, :NKT * BP])

                # ---- q_b (norm weight folded in) per head -> nope/pe ----
                ps_n = qps("qbn")
                ps_p = qps("qbp")
                for h in range(H):
                    wt = wsp.tile([128, NKT, DN + DR], F32, tag="wqb")
                    nc.sync.dma_start(
                        wt[:, :, :],
                        w_qb[:, h * (DN + DR):(h + 1) * (DN + DR)]
                        .rearrange("(t p) m -> p t m", p=128))
                    for t in range(NKT):
                        nc.tensor.matmul(ps_n[:, h * BP:(h + 1) * BP],
                                         wt[:, t, :DN], qanT[:, t, :],
                                         start=(t == 0), stop=(t == NKT - 1))
                    for t in range(NKT):
                        nc.tensor.matmul(ps_p[:64, h * BP:(h + 1) * BP],
                                         wt[:, t, DN:], qanT[:, t, :],
                                         start=(t == 0), stop=(t == NKT - 1))
                qnopeT = qsb.tile([128, H, BP], F32)
                nc.any.tensor_copy(qnopeT[:, :, :],
                                   ps_n[:, :H * BP]
                                   .rearrange("p (h b) -> p h b", h=H))
                qpe_raw = qsb.tile([64, H, BP], F32)
                nc.any.tensor_copy(qpe_raw[:, :, :],
                                   ps_p[:64, :H * BP]
                                   .rearrange("p (h b) -> p h b", h=H))

                # ---- rope(q_pe) as matmul with per-batch rotation ----
                ps_r = qps("rope")
                for h in range(H):
                    for b in range(BP):
                        nc.tensor.matmul(
                            ps_r[:64, h * BP + b:h * BP + b + 1],
                            rt_sb[:, b, :], qpe_raw[:, h, b:b + 1],
                            start=True, stop=True)
                qpeT = qsb.tile([64, H, BP], F32)
                nc.any.tensor_copy(qpeT[:, :, :],
                                   ps_r[:64, :H * BP]
                                   .rearrange("p (h b) -> p h b", h=H))

                # ---- absorb q_nope through w_kc: qabsT [128, 4, H, BP] ----
                ps_a = [qps(f"abs{c}") for c in range(4)]
                for h in range(H):
                    kt_ = wsp.tile([128, KL], F32, tag="wkc")
                    nc.sync.dma_start(kt_[:, :], w_kc[h, :, :])
                    for c in range(4):
                        nc.tensor.matmul(ps_a[c][:, h * BP:(h + 1) * BP],
                                         kt_[:, c * 128:(c + 1) * 128],
                                         qnopeT[:, h, :],
                                         start=True, stop=True)
                qabsT = qsb.tile([128, 4, H, BP], F32)
                for c in range(4):
                    nc.any.tensor_copy(qabsT[:, c, :, :],
                                       ps_a[c][:, :H * BP]
                                       .rearrange("p (h b) -> p h b", h=H))

            # ================= attention =================
            wvc_res = qsb.tile([128, H, 4, DV], F32)
            nc.sync.dma_start(
                wvc_res[:, :, :, :],
                w_vc[:, :, :].rearrange("h (c p) v -> p h c v", p=128))
            ctxT = qsb.tile([128, 4, H, BP], F32)
            with (
                tc.tile_pool(name="pssc", bufs=2, space="PSUM") as pssc,
                tc.tile_pool(name="psctx", bufs=2, space="PSUM") as psctx,
                tc.tile_pool(name="pssum", bufs=1, space="PSUM") as pssum,
                tc.tile_pool(name="psctt", bufs=1, space="PSUM") as psctt,
                tc.tile_pool(name="pstr", bufs=2, space="PSUM") as pstr,
            ):
                sums = pssum.tile([16, BP], F32, tag="sums")

                def attn_seq(lb, ctx_ps):
                    seq_ctr = [None]
                    for st in range(ST):
                        s0 = st * 512
                        ctl = ctlp.tile([128, 4, 512], F32, tag="ctl")
                        nc.sync.dma_start(
                            ctl[:, :, :],
                            cacheT_l[lb, :, s0:s0 + 512]
                            .rearrange("(t p) s -> p t s", p=128))
                        if st == 0:
                            ctr_seq = ctrp.tile([64, S], F32, tag="ctr")
                            nc.sync.dma_start(ctr_seq[:, :],
                                              cacheT_r[lb, :, :])
                            seq_ctr[0] = ctr_seq
                        ctr = seq_ctr[0][:, s0:s0 + 512]
                        sc = pssc.tile([128, 4 * H], F32, tag="sc")
                        for i in range(4):
                            for c in range(4):
                                nc.tensor.matmul(
                                    sc[:, i * H:(i + 1) * H],
                                    ctl[:, c, i * 128:(i + 1) * 128],
                                    qabsT[:, c, :, lb],
                                    start=(c == 0), stop=False)
                            nc.tensor.matmul(
                                sc[:, i * H:(i + 1) * H],
                                ctr[:, i * 128:(i + 1) * 128],
                                qpeT[:, :, lb], start=False, stop=True)
                        eT = etp.tile([128, 4 * H], F32, tag="eT")
                        nc.scalar.activation(eT[:, :], sc[:, :], EXP,
                                             scale=SCALE)
                        for i in range(4):
                            # natural-layout chunk: PE-transpose the resident
                            # [c, s] tile for TRF of 4 chunks, stream the
                            # rest from the host natural layout -- balances
                            # the HBM-read saving against PE transpose cost
                            natc = natp.tile([128, KL], F32, tag="nat")
                            if i < TRF:
                                ps_tr = pstr.tile([128, KL], F32, tag="tr")
                                for c in range(4):
                                    nc.tensor.transpose(
                                        ps_tr[:, c * 128:(c + 1) * 128],
                                        ctl[:, c, i * 128:(i + 1) * 128],
                                        ident[:, :])
                                nc.scalar.copy(natc[:, :], ps_tr[:, :])
                            else:
                                nc.sync.dma_start(
                                    natc[:, :],
                                    cache_nat[lb,
                                              s0 + i * 128:s0 + (i + 1) * 128,
                                              :])
                            nc.tensor.matmul(
                                ctx_ps[:16, :], eT[:, i * H:(i + 1) * H],
                                natc[:, :],
                                start=(st == 0 and i == 0),
                                stop=(st == ST - 1 and i == 3))
                            nc.tensor.matmul(
                                sums[:16, lb:lb + 1],
                                eT[:, i * H:(i + 1) * H], ones_col[:, :1],
                                start=(st == 0 and i == 0),
                                stop=(st == ST - 1 and i == 3))

                for lb in range(BP):
                    ctx_ps = psctx.tile([16, KL], F32, tag="ctx",
                                        name=f"ctx{lb}")
                    attn_seq(lb, ctx_ps)
                    rec = smp2.tile([16, 1], F32, tag="rec")
                    nc.vector.reciprocal(rec[:, :], sums[:16, lb:lb + 1])
                    ctxn = smp2.tile([16, KL], F32, tag="ctxn")
                    nc.vector.tensor_scalar_mul(ctxn[:, :], ctx_ps[:16, :],
                                                rec[:, :1])
                    ps_ct = psctt.tile([128, 4 * H], F32, tag="ctxT")
                    for c in range(4):
                        nc.tensor.transpose(ps_ct[:, c * H:(c + 1) * H],
                                            ctxn[:16, c * 128:(c + 1) * 128],
                                            ident[:16, :16])
                    nc.any.tensor_copy(
                        ctxT[:, :, :, lb],
                        ps_ct[:, :].rearrange("p (c h) -> p c h", c=4))

                # ---- un-absorb values: ovT [128 v, H, BP] ----
                ps_v = pssum.tile([128, H * BP], F32, tag="sums",
                                  name="ps_v")
                for h in range(H):
                    for c in range(4):
                        nc.tensor.matmul(ps_v[:, h * BP:(h + 1) * BP],
                                         wvc_res[:, h, c, :], ctxT[:, c, h, :],
                                         start=(c == 0), stop=(c == 3))
                ovT = qsb.tile([128, H, BP], F32)
                nc.any.tensor_copy(ovT[:, :, :],
                                   ps_v[:, :]
                                   .rearrange("p (h b) -> p h b", h=H))

            # ================= output projection =================
            with (
                tc.tile_pool(name="psor", bufs=1, space="PSUM") as psor,
                tc.tile_pool(name="psot", bufs=1, space="PSUM") as psot,
                tc.tile_pool(name="psoo", bufs=2, space="PSUM") as psoo,
            ):
                if tp:
                    # ovT -> rows [4, 2048] -> AllGather -> [32, 2048] -> T
                    ps_rows = psor.tile([BP, H * DV], F32, tag="ovr")
                    for h in range(H):
                        nc.tensor.transpose(
                            ps_rows[:BP, h * DV:(h + 1) * DV],
                            ovT[:, h, :], ident[:, :])
                    ov_rows = smp.tile([BP, H * DV], F32, tag="ovrows")
                    nc.any.tensor_copy(ov_rows[:, :], ps_rows[:BP, :])
                    agv_in = dramp.tile([BP, H * DV], F32)
                    agv_out = dramp.tile([B, H * DV], F32)
                    nc.sync.dma_start(agv_in[:, :], ov_rows[:, :])
                    if fake_coll:
                        nc.sync.dma_start(agv_out[0:BP, :], agv_in[:, :])
                    else:
                        nc.gpsimd.collective_compute(
                            "AllGather", BYPASS, replica_groups=rg,
                            ins=[agv_in.opt()], outs=[agv_out.opt()])
                    ov32 = smp.tile([B, H * DV], F32, tag="ov32")
                    nc.sync.dma_start(ov32[:, :], agv_out[:, :])
                    ps_tt = psot.tile([128, 16 * B], F32, tag="ovtt")
                    for kt in range(16):
                        nc.tensor.transpose(
                            ps_tt[:, kt * B:(kt + 1) * B],
                            ov32[:B, kt * 128:(kt + 1) * 128], ident[:B, :B])
                    ovT_f = qsb.tile([128, 16, B], F32)
                    nc.any.tensor_copy(ovT_f[:, :, :],
                                       ps_tt[:, :]
                                       .rearrange("p (k b) -> p k b", k=16))
                    lhs_o, NBO = ovT_f, B
                else:
                    lhs_o, NBO = ovT, BP

                out_sb = qsb.tile([NBO, HO], F32)
                for n0 in range(0, HO, 512):
                    nn = min(512, HO - n0)
                    wo_t = wop.tile([128, 16, 512], F32, tag="wo")
                    nc.sync.dma_start(
                        wo_t[:, :, :nn],
                        w_o[:, n0:n0 + nn]
                        .rearrange("(t p) n -> p t n", p=128))
                    ps_o = psoo.tile([NBO, 512], F32, tag="oproj")
                    for kt in range(16):
                        nc.tensor.matmul(ps_o[:, :nn], lhs_o[:, kt, :],
                                         wo_t[:, kt, :nn],
                                         start=(kt == 0), stop=(kt == 15))
                    nc.any.tensor_copy(out_sb[:, n0:n0 + nn], ps_o[:, :nn])
                nc.sync.dma_start(out[:, :], out_sb[:, :])

    nc.compile()
    return nc


# ----------------------------- host wrapper ------------------------------


def _prep_in_maps(inputs, S, n_cores, tp):
    hidden = np.asarray(inputs["hidden_states"], np.float32)
    pos = np.asarray(inputs["positions"], np.int32)
    w_qkv_a = np.asarray(inputs["w_qkv_a"], np.float32)
    q_a_norm_w = np.asarray(inputs["q_a_norm_w"], np.float32)
    w_q_b = np.asarray(inputs["w_q_b"], np.float32)
    kv_a_norm_w = np.asarray(inputs["kv_a_norm_w"], np.float32)
    w_kc = np.asarray(inputs["w_kc"], np.float32)
    w_vc = np.asarray(inputs["w_vc"], np.float32)
    w_o = np.asarray(inputs["w_o"], np.float32)
    cache_l = np.asarray(inputs["kv_cache_latent"], np.float32)
    cache_r = np.asarray(inputs["kv_cache_rope"], np.float32)

    # current-token cache update (host)
    latent = hidden @ w_qkv_a[:, QL:QL + KL]
    k_pe = hidden @ w_qkv_a[:, QL + KL:]
    latent_n = _rmsnorm_np(latent, kv_a_norm_w)
    k_pe_r = _rope_np(k_pe.astype(np.float32), pos)
    cache_l = cache_l.copy()
    cache_r = cache_r.copy()
    cache_l[:, -1, :] = latent_n
    cache_r[:, -1, :] = k_pe_r
    cacheT_l = np.ascontiguousarray(cache_l.transpose(0, 2, 1))
    cacheT_r = np.ascontiguousarray(cache_r.transpose(0, 2, 1))

    hiddenT = np.ascontiguousarray(hidden.T)
    w_qb_eff = np.ascontiguousarray(q_a_norm_w[:, None] * w_q_b)
    RT = _rope_RT(pos)
    w_qa_q = np.ascontiguousarray(w_qkv_a[:, :QL])

    in_maps = []
    for k in range(n_cores):
        b0 = k * BP
        if tp:
            k0 = k * (HID // n_cores)
            k1 = (k + 1) * (HID // n_cores)
            m = {
                "hT": np.ascontiguousarray(hiddenT[k0:k1, :]),
                "w_qa": np.ascontiguousarray(w_qa_q[k0:k1, :]),
                "w_o": np.ascontiguousarray(
                    w_o[:, k * (HID // n_cores):(k + 1) * (HID // n_cores)]),
            }
        else:
            m = {
                "hT": np.ascontiguousarray(hiddenT[:, b0:b0 + BP]),
                "w_qa": w_qa_q,
                "w_o": np.ascontiguousarray(w_o),
            }
        m.update({
            "w_qb": w_qb_eff,
            "w_kc": np.ascontiguousarray(w_kc),
            "w_vc": np.ascontiguousarray(w_vc),
            "cache_nat": np.ascontiguousarray(cache_l[b0:b0 + BP, :S, :]),
            "cacheT_l": np.ascontiguousarray(cacheT_l[b0:b0 + BP, :, :S]),
            "cacheT_r": np.ascontiguousarray(cacheT_r[b0:b0 + BP, :, :S]),
            "ropeRT": np.ascontiguousarray(RT[b0:b0 + BP]),
        })
        in_maps.append(m)
    return in_maps


def _unshard(results, tp):
    if tp:
        return np.concatenate([results[k]["out"] for k in range(N_CORES)],
                              axis=1)
    return np.concatenate([results[k]["out"] for k in range(N_CORES)], axis=0)


def run(inputs, S=4096, trace=False):
    key = (S, N_CORES, TP)
    if key not in _CACHE:
        _CACHE[key] = _build(S, N_CORES, TP)
    nc = _CACHE[key]
    in_maps = _prep_in_maps(inputs, S, N_CORES, TP)
    res = bass_utils.run_bass_kernel_spmd(
        nc, in_maps, core_ids=list(range(N_CORES)), trace=trace)
    return _unshard(res.results, TP), res


def kernel(**inputs) -> np.ndarray:
    out, _ = run(inputs)
    return out.astype(np.float32)



# revision 12
# speedup vs baseline: 2.4572x; 2.4572x over previous
"""DeepseekV2 MLA decode attention on 8 Trainium2 NeuronCores.

Strategy (single SPMD launch, identical program on all cores):

  - Attention is batch-sharded: core k owns sequences 4k..4k+4. The latent
    KV cache is fed in bf16 in the transposed [c, s] layout (for the score
    matmul, which contracts c); the natural [s, c] chunks needed by the
    context matmul (contracts s) are produced on-chip by PE transposes of
    the resident [c, s] tile (TRF chunks per 128-row block; the rest can
    stream from a host-prepared natural layout).
  - All matmul operands are bf16 (1 PE cycle/row vs 4 for fp32, and half
    the HBM bytes); accumulation stays fp32 in PSUM. Matmuls are oriented
    so the large cache tiles are the stationary operand and the moving
    operand is small (16 heads / 4 sequences).
  - The context matmul produces ctx TRANSPOSED ([c, h] per 128-chunk of c,
    moving dim = 16 heads) accumulating over the whole sequence in PSUM.
    Softmax normalization (per-head scalars, i.e. along the free dim) is
    applied via a ones-outer-product broadcast matmul + one vector mult.
  - w_qkv_a's q columns are column-sharded: each core computes its own 192
    q_a columns at full 5120 contraction, which is exactly the k-slice its
    w_q_b shard (K-sharded 192 rows) consumes. Only the rmsnorm sum of
    squares needs an AllReduce ([1, 32] fp32). The q_b partial products
    ([32, 3072] rows) are ReduceScattered so each core gets its 4
    sequences' q. w_o is column-sharded behind an AllGather of the
    per-core attention outputs (bf16).
  - The current-token cache update (rmsnorm latent / roped k_pe written at
    slot S-1) is applied on the host while building the cache layouts;
    rope rotation matrices for q are host-prepared per batch row.
"""

import sys

sys.path.insert(0, "/opt/trn_rl_repo")

import ml_dtypes
import numpy as np

import concourse.bacc as bacc
import concourse.mybir as mybir
import concourse.tile as tile
from concourse import bass_utils
from concourse.masks import make_identity

F32 = mybir.dt.float32
BF16 = mybir.dt.bfloat16
NPBF = ml_dtypes.bfloat16
ADD = mybir.AluOpType.add
MULT = mybir.AluOpType.mult
BYPASS = mybir.AluOpType.bypass
EXP = mybir.ActivationFunctionType.Exp
SQRT = mybir.ActivationFunctionType.Sqrt
SQUARE = mybir.ActivationFunctionType.Square

B, HID, H = 32, 5120, 16
DN, DR, DV = 128, 64, 128
QL, KL = 1536, 512
BASE = 10000.0
EPS = 1e-6
SCALE = float((DN + DR) ** -0.5)

N_CORES = 8
BP = B // N_CORES        # sequences per core
QS = QL // N_CORES       # q_a columns / w_q_b rows per core (192)
HO = HID // N_CORES      # output columns per core (640)
KTH = HID // 128         # hidden k-tiles (40)
TP = True                # kept for test.py compatibility
TRF = 4                  # i-chunks per 128-row block transposed on-chip (0-4)

_CACHE = {}


# ----------------------------- host math ---------------------------------


def _rmsnorm_np(x, w):
    ms = np.mean(x * x, axis=-1, keepdims=True, dtype=np.float32)
    return (x * (1.0 / np.sqrt(ms + EPS)) * w).astype(np.float32)


def _rope_np(x, pos):
    d = x.shape[-1]
    inv = (1.0 / (BASE ** (np.arange(0, d, 2, dtype=np.float32) / d))).astype(
        np.float32
    )
    fr = pos.astype(np.float32)[:, None] * inv
    cos, sin = np.cos(fr).astype(np.float32), np.sin(fr).astype(np.float32)
    out = np.empty_like(x)
    out[..., 0::2] = x[..., 0::2] * cos - x[..., 1::2] * sin
    out[..., 1::2] = x[..., 1::2] * cos + x[..., 0::2] * sin
    return out.astype(np.float32)


def _rope_RT(pos):
    """Per-batch transposed rotation matrices (lhsT for rope-as-matmul)."""
    inv = (1.0 / (BASE ** (np.arange(0, DR, 2, dtype=np.float32) / DR))).astype(
        np.float32
    )
    fr = pos.astype(np.float32)[:, None] * inv
    cos, sin = np.cos(fr).astype(np.float32), np.sin(fr).astype(np.float32)
    R = np.zeros((B, DR, DR), np.float32)
    j = np.arange(DR // 2)
    bi = np.arange(B)[:, None]
    R[bi, 2 * j, 2 * j] = cos
    R[bi, 2 * j, 2 * j + 1] = -sin
    R[bi, 2 * j + 1, 2 * j] = sin
    R[bi, 2 * j + 1, 2 * j + 1] = cos
    return np.ascontiguousarray(R.transpose(0, 2, 1))


# ----------------------------- device program ----------------------------


def _copy_eng(nc, idx):
    """Rotate PSUM->SBUF copies across DVE / ACT (Pool cannot read PSUM)."""
    return [nc.vector.tensor_copy, nc.scalar.copy][idx % 2]


def _build(S, n_cores, tp, fake_coll=False, trf=TRF):
    nc = bacc.Bacc("TRN2", target_bir_lowering=False, debug=False,
                   enable_asserts=False, num_devices=n_cores)
    ST = S // 512
    rg = [list(range(n_cores))]
    NSTR = 4 - trf           # i-chunks streamed from host natural layout

    ctl = nc.dram_tensor("ctl", [BP * ST * 128, 2048], BF16,
                         kind="ExternalInput")
    if NSTR:
        nat = nc.dram_tensor("nat", [BP * ST * NSTR * 128, KL], BF16,
                             kind="ExternalInput")
    ctr = nc.dram_tensor("ctr", [BP * 64, S], BF16, kind="ExternalInput")
    hT = nc.dram_tensor("hT", [128, KTH * B], BF16, kind="ExternalInput")
    w_qa = nc.dram_tensor("w_qa", [128, KTH * QS], BF16, kind="ExternalInput")
    w_qb = nc.dram_tensor("w_qb", [128, 2 * H * (DN + DR)], BF16,
                          kind="ExternalInput")
    w_kc = nc.dram_tensor("w_kc", [128, H * KL], BF16, kind="ExternalInput")
    w_vc = nc.dram_tensor("w_vc", [128, H * KL], BF16, kind="ExternalInput")
    w_o = nc.dram_tensor("w_o", [128, 16 * HO], BF16, kind="ExternalInput")
    ropeRT = nc.dram_tensor("ropeRT", [BP, DR, DR], F32, kind="ExternalInput")
    out = nc.dram_tensor("out", [128, 5 * B], F32, kind="ExternalOutput")

    HDR = H * (DN + DR)  # 3072

    with tile.TileContext(nc) as tc:
        with (
            tc.tile_pool(name="const", bufs=1) as cp,
            tc.tile_pool(name="qsb", bufs=1) as qsb,
            tc.tile_pool(name="dram", bufs=1, space="DRAM") as dramp,
            tc.tile_pool(name="ctl", bufs=4) as ctlp,
            tc.tile_pool(name="ctr", bufs=2) as ctrp,
            tc.tile_pool(name="nat", bufs=3) as natp,
            tc.tile_pool(name="et", bufs=3) as etp,
            tc.tile_pool(name="small", bufs=2) as smp,
        ):
            ones_col = cp.tile([128, 1], F32)
            nc.any.memset(ones_col, 1.0)
            ones_row = cp.tile([1, 128], F32)
            nc.any.memset(ones_row, 1.0)
            ones_bf = cp.tile([128, 1], BF16)
            nc.any.memset(ones_bf, 1.0)
            eps_t = cp.tile([1, 1], F32)
            nc.any.memset(eps_t, EPS)
            identB = cp.tile([128, 128], BF16)
            make_identity(nc, identB[:, :])
            identF = cp.tile([16, 16], F32)
            make_identity(nc, identF[:, :])
            rt_sb = cp.tile([DR, BP, DR], F32)
            nc.sync.dma_start(rt_sb[:, :, :],
                              ropeRT[:, :, :].rearrange("b k m -> k b m"))
            hT_sb = cp.tile([128, KTH, B], BF16)
            nc.sync.dma_start(hT_sb[:, :, :],
                              hT[:, :].rearrange("p (t b) -> p t b", t=KTH))
            w_kc_sb = qsb.tile([128, H, KL], BF16)
            nc.sync.dma_start(w_kc_sb[:, :, :],
                              w_kc[:, :].rearrange("p (h c) -> p h c", h=H))

            # ================= q path =================
            qaTb = qsb.tile([128, B], BF16)
            qaTb2 = qsb.tile([64, B], BF16)
            with tc.tile_pool(name="psq1", bufs=1, space="PSUM") as psq1:
                w_qa_sb = qsb.tile([128, KTH, QS], BF16)
                nc.sync.dma_start(
                    w_qa_sb[:, :, :],
                    w_qa[:, :].rearrange("p (t m) -> p t m", t=KTH))

                # ---- qkv_a q-slice, transposed: my 192 cols for all 32 ----
                psA = psq1.tile([128, B], F32, name="psA")
                psB = psq1.tile([64, B], F32, name="psB")
                for kt in range(KTH):
                    nc.tensor.matmul(psA[:, :], w_qa_sb[:, kt, :128],
                                     hT_sb[:, kt, :],
                                     start=(kt == 0), stop=(kt == KTH - 1))
                for kt in range(KTH):
                    nc.tensor.matmul(psB[:, :], w_qa_sb[:, kt, 128:],
                                     hT_sb[:, kt, :],
                                     start=(kt == 0), stop=(kt == KTH - 1))

                # ---- partial sum of squares -> AllReduce -> rinv ----
                sqA = smp.tile([128, B], F32, tag="sqA")
                nc.scalar.activation(sqA[:, :], psA[:, :], SQUARE)
                sqB = smp.tile([64, B], F32, tag="sqB")
                nc.scalar.activation(sqB[:, :], psB[:, :], SQUARE)
                ps_ss = psq1.tile([1, B], F32, name="ps_ss")
                nc.tensor.matmul(ps_ss[:, :], ones_col[:, :1], sqA[:, :],
                                 start=True, stop=False)
                nc.tensor.matmul(ps_ss[:, :], ones_col[:64, :1], sqB[:, :],
                                 start=False, stop=True)
                ss_sb = smp.tile([1, B], F32, tag="ss")
                nc.scalar.copy(ss_sb[:, :], ps_ss[:, :])
                ar_in = dramp.tile([1, B], F32)
                ar_out = dramp.tile([1, B], F32)
                nc.sync.dma_start(ar_in[:, :], ss_sb[:, :])
                if fake_coll:
                    nc.sync.dma_start(ar_out[:, :], ar_in[:, :])
                else:
                    nc.gpsimd.collective_compute(
                        "AllReduce", ADD, replica_groups=rg,
                        ins=[ar_in.opt()], outs=[ar_out.opt()])
                ss_full = smp.tile([1, B], F32, tag="ssf")
                nc.sync.dma_start(ss_full[:, :], ar_out[:, :])
                rms = smp.tile([1, B], F32, tag="rms")
                nc.scalar.activation(rms[:, :], ss_full[:, :], SQRT,
                                     bias=eps_t[:1, :1], scale=1.0 / QL)
                rinv = smp.tile([1, B], F32, tag="rinv")
                nc.vector.reciprocal(rinv[:, :], rms[:, :])

                # ---- rmsnorm scale along free dim via broadcast matmul ----
                bc_ps = psq1.tile([128, B], F32, name="bc_ps")
                nc.tensor.matmul(bc_ps[:, :], ones_row[:1, :], rinv[:1, :],
                                 start=True, stop=True)
                bc_sb = smp.tile([128, B], F32, tag="bc")
                nc.scalar.copy(bc_sb[:, :], bc_ps[:, :])
                nc.vector.tensor_tensor(qaTb[:, :], psA[:, :], bc_sb[:, :],
                                        MULT)
                nc.vector.tensor_tensor(qaTb2[:, :], psB[:, :],
                                        bc_sb[:64, :], MULT)

            # ---- q_b partials (rows) -> ReduceScatter ----
            qrows_sb = qsb.tile([B, HDR], F32)
            with tc.tile_pool(name="psq2", bufs=2, space="PSUM") as psq2:
                w_qb_sb = qsb.tile([128, 2, HDR], BF16)
                nc.sync.dma_start(
                    w_qb_sb[:, :, :],
                    w_qb[:, :].rearrange("p (t m) -> p t m", t=2))
                for j in range(HDR // 512):
                    ps_q = psq2.tile([B, 512], F32, tag="q",
                                     name=f"ps_q{j}")
                    nc.tensor.matmul(ps_q[:, :], qaTb[:, :],
                                     w_qb_sb[:, 0, j * 512:(j + 1) * 512],
                                     start=True, stop=False)
                    nc.tensor.matmul(ps_q[:, :], qaTb2[:, :],
                                     w_qb_sb[:64, 1, j * 512:(j + 1) * 512],
                                     start=False, stop=True)
                    _copy_eng(nc, j)(qrows_sb[:, j * 512:(j + 1) * 512],
                                     ps_q[:, :])
            rs_in = dramp.tile([B, HDR], F32)
            rs_out = dramp.tile([BP, HDR], F32)
            nc.sync.dma_start(rs_in[:, :], qrows_sb[:, :])
            if fake_coll:
                nc.sync.dma_start(rs_out[:, :], rs_in[0:BP, :])
            else:
                nc.gpsimd.collective_compute(
                    "ReduceScatter", ADD, replica_groups=rg,
                    ins=[rs_in.opt()], outs=[rs_out.opt()])
            qr = qsb.tile([BP, HDR], F32)
            nc.sync.dma_start(qr[:, :], rs_out[:, :])

            # ---- transpose to head layouts, rope, absorb ----
            qpeT = qsb.tile([64, H, BP], BF16)
            qabsT = qsb.tile([128, 4, H, BP], BF16)
            with tc.tile_pool(name="psq3", bufs=1, space="PSUM") as psq3:
                qn_ps = psq3.tile([128, H, BP], F32, name="qn_ps")
                qp_ps = psq3.tile([64, H, BP], F32, name="qp_ps")
                for h in range(H):
                    o = h * (DN + DR)
                    nc.tensor.transpose(qn_ps[:, h, :],
                                        qr[:BP, o:o + DN], identF[:BP, :BP])
                    nc.tensor.transpose(qp_ps[:, h, :],
                                        qr[:BP, o + DN:o + DN + DR],
                                        identF[:BP, :BP])
                qnopeT = qsb.tile([128, H, BP], BF16)
                nc.vector.tensor_copy(qnopeT[:, :, :], qn_ps[:, :, :])
                qpe_raw = smp.tile([64, H, BP], F32, tag="qperaw")
                nc.scalar.copy(qpe_raw[:, :, :], qp_ps[:, :, :])
                rope_ps = psq3.tile([64, BP, H], F32, name="rope_ps")
                for b in range(BP):
                    nc.tensor.matmul(rope_ps[:, b, :], rt_sb[:, b, :],
                                     qpe_raw[:, :, b], start=True, stop=True)
                nc.vector.tensor_copy(
                    qpeT[:, :, :],
                    rope_ps[:, :, :].rearrange("p b h -> p h b"))
                qabs_ps = psq3.tile([128, 4, H, BP], F32, name="qabs_ps")
                for h in range(H):
                    for c in range(4):
                        nc.tensor.matmul(qabs_ps[:, c, h, :],
                                         w_kc_sb[:, h, c * 128:(c + 1) * 128],
                                         qnopeT[:, h, :],
                                         start=True, stop=True)
                nc.scalar.copy(qabsT[:, :, :, :], qabs_ps[:, :, :, :])

            # ================= attention =================
            ctxTn = qsb.tile([128, 4, H, BP], BF16)
            with (
                tc.tile_pool(name="pssc", bufs=2, space="PSUM") as pssc,
                tc.tile_pool(name="pstr", bufs=2, space="PSUM") as pstr,
                tc.tile_pool(name="psctx", bufs=2, space="PSUM") as psctx,
                tc.tile_pool(name="pssum", bufs=1, space="PSUM") as pssum,
                tc.tile_pool(name="psn", bufs=1, space="PSUM") as psn,
                tc.tile_pool(name="ctxa", bufs=2) as ctxap,
            ):
                sums = pssum.tile([16, BP], F32, tag="sums")

                def emit_ctx(lb, st, eT, natc, ctxa):
                    # per-st PSUM groups are sequential (one pending group
                    # per zero region); accumulate across st in SBUF
                    ctx_ps = psctx.tile([128, 4, 16], F32, tag="ctxst",
                                        name=f"cst{lb}_{st}")
                    for c in range(4):
                        for i in range(4):
                            nc.tensor.matmul(
                                ctx_ps[:, c, :],
                                natc[:, i, c * 128:(c + 1) * 128],
                                eT[:, i, :],
                                start=(i == 0), stop=(i == 3))
                    for i in range(4):
                        nc.tensor.matmul(
                            sums[:16, lb:lb + 1], eT[:, i, :],
                            ones_bf[:, :1],
                            start=(st == 0 and i == 0),
                            stop=(st == ST - 1 and i == 3))
                    if st == 0:
                        nc.vector.tensor_copy(ctxa[:, :, :], ctx_ps[:, :, :])
                    else:
                        nc.vector.tensor_tensor(ctxa[:, :, :], ctx_ps[:, :, :],
                                                ctxa[:, :, :], ADD)

                for lb in range(BP):
                    ctr_sb = ctrp.tile([64, S], BF16, tag="ctr")
                    nc.sync.dma_start(ctr_sb[:, :],
                                      ctr[lb * 64:(lb + 1) * 64, :])
                    ctxa = ctxap.tile([128, 4, 16], F32, tag="ctxa",
                                      name=f"ctxa{lb}")
                    pend = None
                    for st in range(ST):
                        ctl_sb = ctlp.tile([128, 4, 512], BF16, tag="ctl")
                        nc.sync.dma_start(
                            ctl_sb[:, :, :],
                            ctl[(lb * ST + st) * 128:(lb * ST + st + 1) * 128,
                                :].rearrange("p (c s) -> p c s", c=4))
                        sc = pssc.tile([128, 4, 16], F32, tag="sc")
                        for i in range(4):
                            for c in range(4):
                                nc.tensor.matmul(
                                    sc[:, i, :],
                                    ctl_sb[:, c, i * 128:(i + 1) * 128],
                                    qabsT[:, c, :, lb],
                                    start=(c == 0), stop=False)
                            s0 = st * 512 + i * 128
                            nc.tensor.matmul(sc[:, i, :],
                                             ctr_sb[:, s0:s0 + 128],
                                             qpeT[:, :, lb],
                                             start=False, stop=True)
                        eT = etp.tile([128, 4, 16], BF16, tag="eT")
                        nc.scalar.activation(eT[:, :, :], sc[:, :, :], EXP,
                                             scale=SCALE)
                        natc = natp.tile([128, 4, KL], BF16, tag="nat")
                        for i in range(trf):
                            tr = pstr.tile([128, KL], BF16, tag="tr")
                            for c in range(4):
                                nc.tensor.transpose(
                                    tr[:, c * 128:(c + 1) * 128],
                                    ctl_sb[:, c, i * 128:(i + 1) * 128],
                                    identB[:, :])
                            _copy_eng(nc, st * 4 + i)(natc[:, i, :], tr[:, :])
                        if NSTR:
                            r0 = (lb * ST + st) * NSTR * 128
                            nc.sync.dma_start(
                                natc[:, trf:, :],
                                nat[r0:r0 + NSTR * 128, :]
                                .rearrange("(i p) c -> p i c", p=128))
                        if pend is not None:
                            emit_ctx(lb, pend[0], pend[1], pend[2], ctxa)
                        pend = (st, eT, natc)
                    emit_ctx(lb, pend[0], pend[1], pend[2], ctxa)

                    # ---- normalize: ctxTn[:, c, :, lb] = ctxp * (1/sums) ----
                    rec = smp.tile([16, 1], F32, tag="rec")
                    nc.vector.reciprocal(rec[:, :], sums[:16, lb:lb + 1])
                    nb = psn.tile([128, 32], F32, tag="nrm", name=f"nrm{lb}")
                    nc.tensor.transpose(nb[:1, 0:16], rec[:16, :1],
                                        identF[:16, :16])
                    recT = smp.tile([1, 16], F32, tag="recT")
                    nc.scalar.copy(recT[:, :], nb[:1, 0:16])
                    nc.tensor.matmul(nb[:, 16:32], ones_row[:1, :],
                                     recT[:1, :], start=True, stop=True)
                    bcn = smp.tile([128, 16], F32, tag="bcnsb")
                    nc.scalar.copy(bcn[:, :], nb[:, 16:32])
                    for c in range(4):
                        nc.vector.tensor_tensor(ctxTn[:, c, :, lb],
                                                ctxa[:, c, :], bcn[:, :],
                                                MULT)

            # ================= tail: unabsorb, AllGather, o_proj ==========
            with (
                tc.tile_pool(name="pst1", bufs=1, space="PSUM") as pst1,
                tc.tile_pool(name="pst2", bufs=1, space="PSUM") as pst2,
            ):
                w_vc_sb = qsb.tile([128, H, 4, DV], BF16)
                nc.sync.dma_start(
                    w_vc_sb[:, :, :, :],
                    w_vc[:, :].rearrange("p (h c v) -> p h c v", h=H, c=4))
                ov_ps = pst1.tile([128, H, BP], F32, name="ov_ps")
                for h in range(H):
                    for c in range(4):
                        nc.tensor.matmul(ov_ps[:, h, :], w_vc_sb[:, h, c, :],
                                         ctxTn[:, c, h, :],
                                         start=(c == 0), stop=(c == 3))
                ov_sb = qsb.tile([128, H, BP], BF16)
                nc.vector.tensor_copy(ov_sb[:, :, :], ov_ps[:, :, :])

                ps_rows = pst2.tile([BP, H * DV], BF16, name="ps_rows")
                for h in range(H):
                    nc.tensor.transpose(ps_rows[:BP, h * DV:(h + 1) * DV],
                                        ov_sb[:, h, :], identB[:, :])
                ov_rows = qsb.tile([BP, H * DV], BF16)
                nc.vector.tensor_copy(ov_rows[:, :H * DV // 2],
                                      ps_rows[:BP, :H * DV // 2])
                nc.scalar.copy(ov_rows[:, H * DV // 2:],
                               ps_rows[:BP, H * DV // 2:])
                ag_in = dramp.tile([BP, H * DV], BF16)
                ag_out = dramp.tile([B, H * DV], BF16)
                nc.sync.dma_start(ag_in[:, :], ov_rows[:, :])
                if fake_coll:
                    nc.sync.dma_start(ag_out[0:BP, :], ag_in[:, :])
                else:
                    nc.gpsimd.collective_compute(
                        "AllGather", BYPASS, replica_groups=rg,
                        ins=[ag_in.opt()], outs=[ag_out.opt()])
                ov32 = qsb.tile([B, H * DV], BF16)
                nc.sync.dma_start(ov32[:, :], ag_out[:, :])

            with (
                tc.tile_pool(name="pst3", bufs=1, space="PSUM") as pst3,
                tc.tile_pool(name="pst4", bufs=1, space="PSUM") as pst4,
            ):
                ps_tt = pst3.tile([128, 16, B], BF16, name="ps_tt")
                for kt in range(16):
                    nc.tensor.transpose(ps_tt[:, kt, :],
                                        ov32[:B, kt * 128:(kt + 1) * 128],
                                        identB[:B, :B])
                ovT_f = qsb.tile([128, 16, B], BF16)
                nc.vector.tensor_copy(ovT_f[:, :, :], ps_tt[:, :, :])

                w_o_sb = qsb.tile([128, 16, HO], BF16)
                nc.sync.dma_start(
                    w_o_sb[:, :, :],
                    w_o[:, :].rearrange("p (t n) -> p t n", t=16))
                out_ps = pst4.tile([128, 5, B], F32, name="out_ps")
                for t in range(5):
                    for kt in range(16):
                        nc.tensor.matmul(
                            out_ps[:, t, :],
                            w_o_sb[:, kt, t * 128:(t + 1) * 128],
                            ovT_f[:, kt, :],
                            start=(kt == 0), stop=(kt == 15))
                out_sb = qsb.tile([128, 5, B], F32)
                nc.scalar.copy(out_sb[:, :, :], out_ps[:, :, :])
                nc.sync.dma_start(
                    out[:, :],
                    out_sb[:, :, :].rearrange("p t b -> p (t b)"))

    nc.compile()
    return nc


# ----------------------------- host wrapper ------------------------------


def _prep_in_maps(inputs, S, n_cores, tp, trf=TRF):
    hidden = np.asarray(inputs["hidden_states"], np.float32)
    pos = np.asarray(inputs["positions"], np.int32)
    w_qkv_a = np.asarray(inputs["w_qkv_a"], np.float32)
    q_a_norm_w = np.asarray(inputs["q_a_norm_w"], np.float32)
    w_q_b = np.asarray(inputs["w_q_b"], np.float32)
    kv_a_norm_w = np.asarray(inputs["kv_a_norm_w"], np.float32)
    w_kc = np.asarray(inputs["w_kc"], np.float32)
    w_vc = np.asarray(inputs["w_vc"], np.float32)
    w_o = np.asarray(inputs["w_o"], np.float32)
    cache_l = np.asarray(inputs["kv_cache_latent"], np.float32)
    cache_r = np.asarray(inputs["kv_cache_rope"], np.float32)
    ST = S // 512
    NSTR = 4 - trf

    # current-token cache update (host)
    latent = hidden @ w_qkv_a[:, QL:QL + KL]
    k_pe = hidden @ w_qkv_a[:, QL + KL:]
    latent_n = _rmsnorm_np(latent, kv_a_norm_w)
    k_pe_r = _rope_np(k_pe.astype(np.float32), pos)
    cache_l = cache_l.copy()
    cache_r = cache_r.copy()
    cache_l[:, -1, :] = latent_n
    cache_r[:, -1, :] = k_pe_r
    cache_l_b = cache_l[:, :S, :].astype(NPBF)
    cache_r_b = cache_r[:, :S, :].astype(NPBF)

    hiddenT_b = np.ascontiguousarray(
        hidden.T.reshape(KTH, 128, B).transpose(1, 0, 2)).astype(NPBF)
    w_qb_eff = (q_a_norm_w[:, None] * w_q_b).astype(np.float32)
    RT = _rope_RT(pos)
    w_qa_q = w_qkv_a[:, :QL]
    w_kc_b = np.ascontiguousarray(
        w_kc.transpose(1, 0, 2)).astype(NPBF)            # [128, H, KL]
    w_vc_b = np.ascontiguousarray(
        w_vc.reshape(H, 4, 128, DV).transpose(2, 0, 1, 3)).astype(NPBF)

    in_maps = []
    for k in range(n_cores):
        b0 = k * BP
        cl = cache_l[b0:b0 + BP, :S, :]                  # fp32 view
        # transposed layout [b, st, p(c%128), ct, s]
        ctlT = (cl.transpose(0, 2, 1)
                .reshape(BP, 4, 128, ST, 512)
                .transpose(0, 3, 2, 1, 4))
        ctl_h = np.ascontiguousarray(ctlT).astype(NPBF).reshape(
            BP * ST * 128, 2048)
        ctr_h = np.ascontiguousarray(
            cache_r_b[b0:b0 + BP].transpose(0, 2, 1)).reshape(BP * 64, S)
        wqa_h = np.ascontiguousarray(
            w_qa_q[:, k * QS:(k + 1) * QS]
            .reshape(KTH, 128, QS).transpose(1, 0, 2)).astype(NPBF)
        wqb_pad = np.zeros((256, H * (DN + DR)), np.float32)
        wqb_pad[:QS] = w_qb_eff[k * QS:(k + 1) * QS]
        wqb_h = np.ascontiguousarray(
            wqb_pad.reshape(2, 128, -1).transpose(1, 0, 2)).astype(NPBF)
        wo_h = np.ascontiguousarray(
            w_o[:, k * HO:(k + 1) * HO]
            .reshape(16, 128, HO).transpose(1, 0, 2)).astype(NPBF)
        m = {
            "ctl": ctl_h,
            "ctr": np.ascontiguousarray(ctr_h),
            "hT": hiddenT_b.reshape(128, KTH * B),
            "w_qa": wqa_h.reshape(128, KTH * QS),
            "w_qb": wqb_h.reshape(128, -1),
            "w_kc": w_kc_b.reshape(128, H * KL),
            "w_vc": w_vc_b.reshape(128, H * KL),
            "w_o": wo_h.reshape(128, 16 * HO),
            "ropeRT": np.ascontiguousarray(RT[b0:b0 + BP]),
        }
        if NSTR:
            nat_h = (cache_l_b[b0:b0 + BP]
                     .reshape(BP, ST, 4, 128, KL)[:, :, trf:, :, :])
            m["nat"] = np.ascontiguousarray(nat_h).reshape(
                BP * ST * NSTR * 128, KL)
        in_maps.append(m)
    return in_maps


def _unshard(results, tp):
    cols = []
    for k in range(N_CORES):
        o = results[k]["out"].reshape(128, 5, B)
        cols.append(o.transpose(2, 1, 0).reshape(B, 5 * 128))
    return np.concatenate(cols, axis=1)


def run(inputs, S=4096, trace=False):
    key = (S, N_CORES, TP, TRF)
    if key not in _CACHE:
        _CACHE[key] = _build(S, N_CORES, TP, trf=TRF)
    nc = _CACHE[key]
    in_maps = _prep_in_maps(inputs, S, N_CORES, TP, trf=TRF)
    res = bass_utils.run_bass_kernel_spmd(
        nc, in_maps, core_ids=list(range(N_CORES)), trace=trace)
    return _unshard(res.results, TP), res


def kernel(**inputs) -> np.ndarray:
    out, _ = run(inputs)
    return out.astype(np.float32)


# revision 14
# speedup vs baseline: 2.4611x; 1.0016x over previous
"""DeepseekV2 MLA decode attention on 8 Trainium2 NeuronCores.

Strategy (single SPMD launch, identical program on all cores):

  - Attention is batch-sharded: core k owns sequences 4k..4k+4. The latent
    KV cache is fed in bf16 in the transposed [c, s] layout (for the score
    matmul, which contracts c); the natural [s, c] chunks needed by the
    context matmul (contracts s) are produced on-chip by PE transposes of
    the resident [c, s] tile (TRF chunks per 128-row block; the rest can
    stream from a host-prepared natural layout).
  - All matmul operands are bf16 (1 PE cycle/row vs 4 for fp32, and half
    the HBM bytes); accumulation stays fp32 in PSUM. Matmuls are oriented
    so the large cache tiles are the stationary operand and the moving
    operand is small (16 heads / 4 sequences).
  - The context matmul produces ctx TRANSPOSED ([c, h] per 128-chunk of c,
    moving dim = 16 heads) accumulating over the whole sequence in PSUM.
    Softmax normalization (per-head scalars, i.e. along the free dim) is
    applied via a ones-outer-product broadcast matmul + one vector mult.
  - w_qkv_a's q columns are column-sharded: each core computes its own 192
    q_a columns at full 5120 contraction, which is exactly the k-slice its
    w_q_b shard (K-sharded 192 rows) consumes. Only the rmsnorm sum of
    squares needs an AllReduce ([1, 32] fp32). The q_b partial products
    ([32, 3072] rows) are ReduceScattered so each core gets its 4
    sequences' q. w_o is column-sharded behind an AllGather of the
    per-core attention outputs (bf16).
  - The current-token cache update (rmsnorm latent / roped k_pe written at
    slot S-1) is applied on the host while building the cache layouts;
    rope rotation matrices for q are host-prepared per batch row.
"""

import sys

sys.path.insert(0, "/opt/trn_rl_repo")

import ml_dtypes
import numpy as np

import concourse.bacc as bacc
import concourse.mybir as mybir
import concourse.tile as tile
from concourse import bass_utils
from concourse.masks import make_identity

F32 = mybir.dt.float32
BF16 = mybir.dt.bfloat16
NPBF = ml_dtypes.bfloat16
ADD = mybir.AluOpType.add
MULT = mybir.AluOpType.mult
BYPASS = mybir.AluOpType.bypass
EXP = mybir.ActivationFunctionType.Exp
SQRT = mybir.ActivationFunctionType.Sqrt
SQUARE = mybir.ActivationFunctionType.Square

B, HID, H = 32, 5120, 16
DN, DR, DV = 128, 64, 128
QL, KL = 1536, 512
BASE = 10000.0
EPS = 1e-6
SCALE = float((DN + DR) ** -0.5)

N_CORES = 8
BP = B // N_CORES        # sequences per core
QS = QL // N_CORES       # q_a columns / w_q_b rows per core (192)
HO = HID // N_CORES      # output columns per core (640)
KTH = HID // 128         # hidden k-tiles (40)
TP = True                # kept for test.py compatibility
TRF = 4                  # i-chunks per 128-row block transposed on-chip (0-4)

_CACHE = {}


# ----------------------------- host math ---------------------------------


def _rmsnorm_np(x, w):
    ms = np.mean(x * x, axis=-1, keepdims=True, dtype=np.float32)
    return (x * (1.0 / np.sqrt(ms + EPS)) * w).astype(np.float32)


def _rope_np(x, pos):
    d = x.shape[-1]
    inv = (1.0 / (BASE ** (np.arange(0, d, 2, dtype=np.float32) / d))).astype(
        np.float32
    )
    fr = pos.astype(np.float32)[:, None] * inv
    cos, sin = np.cos(fr).astype(np.float32), np.sin(fr).astype(np.float32)
    out = np.empty_like(x)
    out[..., 0::2] = x[..., 0::2] * cos - x[..., 1::2] * sin
    out[..., 1::2] = x[..., 1::2] * cos + x[..., 0::2] * sin
    return out.astype(np.float32)


def _rope_RT(pos):
    """Per-batch transposed rotation matrices (lhsT for rope-as-matmul)."""
    inv = (1.0 / (BASE ** (np.arange(0, DR, 2, dtype=np.float32) / DR))).astype(
        np.float32
    )
    fr = pos.astype(np.float32)[:, None] * inv
    cos, sin = np.cos(fr).astype(np.float32), np.sin(fr).astype(np.float32)
    R = np.zeros((B, DR, DR), np.float32)
    j = np.arange(DR // 2)
    bi = np.arange(B)[:, None]
    R[bi, 2 * j, 2 * j] = cos
    R[bi, 2 * j, 2 * j + 1] = -sin
    R[bi, 2 * j + 1, 2 * j] = sin
    R[bi, 2 * j + 1, 2 * j + 1] = cos
    return np.ascontiguousarray(R.transpose(0, 2, 1))


# ----------------------------- device program ----------------------------


def _copy_eng(nc, idx):
    """Rotate PSUM->SBUF copies across DVE / ACT (Pool cannot read PSUM)."""
    return [nc.vector.tensor_copy, nc.scalar.copy][idx % 2]


def _build(S, n_cores, tp, fake_coll=False, trf=TRF):
    nc = bacc.Bacc("TRN2", target_bir_lowering=False, debug=False,
                   enable_asserts=False, num_devices=n_cores)
    ST = S // 512
    rg = [list(range(n_cores))]
    NSTR = 4 - trf           # i-chunks streamed from host natural layout

    ctl = nc.dram_tensor("ctl", [BP * ST * 128, 2048], BF16,
                         kind="ExternalInput")
    if NSTR:
        nat = nc.dram_tensor("nat", [BP * ST * NSTR * 128, KL], BF16,
                             kind="ExternalInput")
    ctr = nc.dram_tensor("ctr", [BP * 64, S], BF16, kind="ExternalInput")
    hT = nc.dram_tensor("hT", [128, KTH * B], BF16, kind="ExternalInput")
    w_qa = nc.dram_tensor("w_qa", [128, KTH * QS], BF16, kind="ExternalInput")
    w_qb = nc.dram_tensor("w_qb", [128, 2 * H * (DN + DR)], BF16,
                          kind="ExternalInput")
    w_kc = nc.dram_tensor("w_kc", [128, H * KL], BF16, kind="ExternalInput")
    w_vc = nc.dram_tensor("w_vc", [128, H * KL], BF16, kind="ExternalInput")
    w_o = nc.dram_tensor("w_o", [128, 16 * HO], BF16, kind="ExternalInput")
    ropeRT = nc.dram_tensor("ropeRT", [BP, DR, DR], F32, kind="ExternalInput")
    out = nc.dram_tensor("out", [128, 5 * B], F32, kind="ExternalOutput")

    HDR = H * (DN + DR)  # 3072

    with tile.TileContext(nc) as tc:
        with (
            tc.tile_pool(name="const", bufs=1) as cp,
            tc.tile_pool(name="qsb", bufs=1) as qsb,
            tc.tile_pool(name="dram", bufs=1, space="DRAM") as dramp,
            tc.tile_pool(name="ctl", bufs=6) as ctlp,
            tc.tile_pool(name="ctr", bufs=2) as ctrp,
            tc.tile_pool(name="nat", bufs=3) as natp,
            tc.tile_pool(name="et", bufs=3) as etp,
            tc.tile_pool(name="small", bufs=2) as smp,
        ):
            ones_col = cp.tile([128, 1], F32)
            nc.any.memset(ones_col, 1.0)
            ones_bf = cp.tile([128, 1], BF16)
            nc.any.memset(ones_bf, 1.0)
            eps_t = cp.tile([128, 1], F32)
            nc.any.memset(eps_t, EPS)
            identB = cp.tile([128, 128], BF16)
            make_identity(nc, identB[:, :])
            rt_sb = cp.tile([DR, BP, DR], F32)
            nc.scalar.dma_start(rt_sb[:, :, :],
                              ropeRT[:, :, :].rearrange("b k m -> k b m"))
            hT_sb = cp.tile([128, KTH, B], BF16)
            nc.scalar.dma_start(hT_sb[:, :, :],
                              hT[:, :].rearrange("p (t b) -> p t b", t=KTH))
            w_kc_sb = qsb.tile([128, H, KL], BF16)
            for jj in range(4):
                nc.scalar.dma_start(
                    w_kc_sb[:, jj * 4:(jj + 1) * 4, :],
                    w_kc[:, jj * 4 * KL:(jj + 1) * 4 * KL]
                    .rearrange("p (h c) -> p h c", h=4))

            # ================= q path =================
            qaTb = qsb.tile([128, B], BF16)
            qaTb2 = qsb.tile([64, B], BF16)
            with tc.tile_pool(name="psq1", bufs=1, space="PSUM") as psq1:
                w_qa_sb = qsb.tile([128, KTH, QS], BF16)
                for jj in range(4):
                    nc.scalar.dma_start(
                        w_qa_sb[:, jj * 10:(jj + 1) * 10, :],
                        w_qa[:, jj * 10 * QS:(jj + 1) * 10 * QS]
                        .rearrange("p (t m) -> p t m", t=10))

                # ---- qkv_a q-slice, transposed: my 192 cols for all 32 ----
                psA = psq1.tile([128, B], F32, name="psA")
                psB = psq1.tile([64, B], F32, name="psB")
                for kt in range(KTH):
                    nc.tensor.matmul(psA[:, :], w_qa_sb[:, kt, :128],
                                     hT_sb[:, kt, :],
                                     start=(kt == 0), stop=(kt == KTH - 1))
                for kt in range(KTH):
                    nc.tensor.matmul(psB[:, :], w_qa_sb[:, kt, 128:],
                                     hT_sb[:, kt, :],
                                     start=(kt == 0), stop=(kt == KTH - 1))

                # ---- partial sum of squares -> tiny ReduceScatter ----
                # (q_a stays unnormalized; 1/rms folds into the post-RS
                # transposes as a diagonal matmul, so this collective runs
                # concurrently with the q_b matmuls + big RS)
                sqA = smp.tile([128, B], F32, tag="sqA")
                nc.scalar.activation(sqA[:, :], psA[:, :], SQUARE)
                sqB = smp.tile([64, B], F32, tag="sqB")
                nc.scalar.activation(sqB[:, :], psB[:, :], SQUARE)
                ps_ss = psq1.tile([1, B], F32, name="ps_ss")
                nc.tensor.matmul(ps_ss[:, :], ones_col[:, :1], sqA[:, :],
                                 start=True, stop=False)
                nc.tensor.matmul(ps_ss[:, :], ones_col[:64, :1], sqB[:, :],
                                 start=False, stop=True)
                ss_sb = smp.tile([1, B], F32, tag="ss")
                nc.scalar.copy(ss_sb[:, :], ps_ss[:, :])
                ar_in = dramp.tile([1, B], F32)
                ar_out = dramp.tile([1, BP], F32)
                nc.scalar.dma_start(ar_in[:, :], ss_sb[:, :])
                if fake_coll:
                    nc.scalar.dma_start(ar_out[:, :], ar_in[:1, :BP])
                else:
                    nc.gpsimd.collective_compute(
                        "ReduceScatter", ADD, replica_groups=rg,
                        ins=[ar_in.opt()], outs=[ar_out.opt()])
                ss4 = smp.tile([BP, 1], F32, tag="ssf")
                nc.scalar.dma_start(ss4[:, :],
                                    ar_out[:, :].rearrange("a b -> b a"))
                rms4 = smp.tile([BP, 1], F32, tag="rms")
                nc.scalar.activation(rms4[:, :], ss4[:, :], SQRT,
                                     bias=eps_t[:BP, :1], scale=1.0 / QL)
                rinv4 = smp.tile([BP, 1], F32, tag="rinv")
                nc.vector.reciprocal(rinv4[:, :], rms4[:, :])
                diag4 = smp.tile([BP, BP], BF16, tag="diag")
                nc.vector.tensor_scalar_mul(diag4[:, :], identB[:BP, :BP],
                                            rinv4[:BP, :1])
                nc.vector.tensor_copy(qaTb[:, :], psA[:, :])
                nc.scalar.copy(qaTb2[:, :], psB[:, :])

            # ---- q_b partials (rows) -> ReduceScatter ----
            qrows_sb = qsb.tile([B, HDR], BF16)
            with tc.tile_pool(name="psq2", bufs=2, space="PSUM") as psq2:
                w_qb_sb = qsb.tile([128, 2, HDR], BF16)
                nc.scalar.dma_start(
                    w_qb_sb[:, :, :],
                    w_qb[:, :].rearrange("p (t m) -> p t m", t=2))
                for j in range(HDR // 512):
                    ps_q = psq2.tile([B, 512], F32, tag="q",
                                     name=f"ps_q{j}")
                    nc.tensor.matmul(ps_q[:, :], qaTb[:, :],
                                     w_qb_sb[:, 0, j * 512:(j + 1) * 512],
                                     start=True, stop=False)
                    nc.tensor.matmul(ps_q[:, :], qaTb2[:, :],
                                     w_qb_sb[:64, 1, j * 512:(j + 1) * 512],
                                     start=False, stop=True)
                    _copy_eng(nc, j)(qrows_sb[:, j * 512:(j + 1) * 512],
                                     ps_q[:, :])
            rs_in = dramp.tile([B, HDR], BF16)
            rs_out = dramp.tile([BP, HDR], BF16)
            nc.scalar.dma_start(rs_in[:, :], qrows_sb[:, :])
            if fake_coll:
                nc.scalar.dma_start(rs_out[:, :], rs_in[0:BP, :])
            else:
                nc.gpsimd.collective_compute(
                    "ReduceScatter", ADD, replica_groups=rg,
                    ins=[rs_in.opt()], outs=[rs_out.opt()])
            qr = qsb.tile([BP, HDR], BF16)
            nc.scalar.dma_start(qr[:, :], rs_out[:, :])

            # ---- transpose to head layouts, rope, absorb ----
            qpeT = qsb.tile([64, H, BP], BF16)
            qabsT = qsb.tile([128, 4, H, BP], BF16)
            with tc.tile_pool(name="psq3", bufs=1, space="PSUM") as psq3:
                qn_ps = psq3.tile([128, H, BP], F32, name="qn_ps")
                qp_ps = psq3.tile([64, H, BP], F32, name="qp_ps")
                for h in range(H):
                    o = h * (DN + DR)
                    nc.tensor.matmul(qn_ps[:, h, :], qr[:BP, o:o + DN],
                                     diag4[:, :], start=True, stop=True)
                    nc.tensor.matmul(qp_ps[:, h, :],
                                     qr[:BP, o + DN:o + DN + DR],
                                     diag4[:, :], start=True, stop=True)
                qnopeT = qsb.tile([128, H, BP], BF16)
                nc.vector.tensor_copy(qnopeT[:, :, :], qn_ps[:, :, :])
                qpe_raw = smp.tile([64, H, BP], F32, tag="qperaw")
                nc.scalar.copy(qpe_raw[:, :, :], qp_ps[:, :, :])
                rope_ps = psq3.tile([64, BP, H], F32, name="rope_ps")
                for b in range(BP):
                    nc.tensor.matmul(rope_ps[:, b, :], rt_sb[:, b, :],
                                     qpe_raw[:, :, b], start=True, stop=True)
                nc.vector.tensor_copy(
                    qpeT[:, :, :],
                    rope_ps[:, :, :].rearrange("p b h -> p h b"))
                qabs_ps = psq3.tile([128, 4, H, BP], F32, name="qabs_ps")
                for h in range(H):
                    for c in range(4):
                        nc.tensor.matmul(qabs_ps[:, c, h, :],
                                         w_kc_sb[:, h, c * 128:(c + 1) * 128],
                                         qnopeT[:, h, :],
                                         start=True, stop=True)
                nc.scalar.copy(qabsT[:, :, :, :], qabs_ps[:, :, :, :])

            # ================= attention =================
            w_vc_sb = qsb.tile([128, H, 4, DV], BF16)
            for jj in range(4):
                nc.scalar.dma_start(
                    w_vc_sb[:, jj * 4:(jj + 1) * 4, :, :],
                    w_vc[:, jj * 4 * KL:(jj + 1) * 4 * KL]
                    .rearrange("p (h c v) -> p h c v", h=4, c=4))
            w_o_sb = qsb.tile([128, 16, HO], BF16)
            for jj in range(4):
                nc.scalar.dma_start(
                    w_o_sb[:, jj * 4:(jj + 1) * 4, :],
                    w_o[:, jj * 4 * HO:(jj + 1) * 4 * HO]
                    .rearrange("p (t n) -> p t n", t=4))
            ctxTn = qsb.tile([128, 4, H, BP], BF16)
            ov_sb = qsb.tile([128, H, BP], BF16)
            with (
                tc.tile_pool(name="pssc", bufs=2, space="PSUM") as pssc,
                tc.tile_pool(name="pstr", bufs=2, space="PSUM") as pstr,
                tc.tile_pool(name="psctx", bufs=2, space="PSUM") as psctx,
                tc.tile_pool(name="pssum", bufs=1, space="PSUM") as pssum,
                tc.tile_pool(name="psn", bufs=1, space="PSUM") as psn,
                tc.tile_pool(name="ctxa", bufs=2) as ctxap,
            ):


                def emit_ctx(lb, st, eT, natc, ctxa, sums):
                    # per-st PSUM groups are sequential (one pending group
                    # per zero region); accumulate across st in SBUF
                    ctx_ps = psctx.tile([128, 4, 16], F32, tag="ctxst",
                                        name=f"cst{lb}_{st}")
                    for c in range(4):
                        for i in range(4):
                            nc.tensor.matmul(
                                ctx_ps[:, c, :],
                                natc[:, i, c * 128:(c + 1) * 128],
                                eT[:, i, :],
                                start=(i == 0), stop=(i == 3))
                    for i in range(4):
                        nc.tensor.matmul(
                            sums[:1, :], ones_bf[:, :1], eT[:, i, :],
                            start=(st == 0 and i == 0),
                            stop=(st == ST - 1 and i == 3))
                    if st == 0:
                        nc.vector.tensor_copy(ctxa[:, :, :], ctx_ps[:, :, :])
                    else:
                        nc.vector.tensor_tensor(ctxa[:, :, :], ctx_ps[:, :, :],
                                                ctxa[:, :, :], ADD)

                for lb in range(BP):
                    ctr_sb = ctrp.tile([64, S], BF16, tag="ctr")
                    nc.sync.dma_start(ctr_sb[:, :],
                                      ctr[lb * 64:(lb + 1) * 64, :])
                    ctxa = ctxap.tile([128, 4, 16], F32, tag="ctxa",
                                      name=f"ctxa{lb}")
                    sums = pssum.tile([1, 16], F32, tag="sums",
                                      name=f"sums{lb}")
                    pend = None
                    for st in range(ST):
                        ctl_sb = ctlp.tile([128, 4, 512], BF16, tag="ctl")
                        nc.sync.dma_start(
                            ctl_sb[:, :, :],
                            ctl[(lb * ST + st) * 128:(lb * ST + st + 1) * 128,
                                :].rearrange("p (c s) -> p c s", c=4))
                        sc = pssc.tile([128, 4, 16], F32, tag="sc")
                        for i in range(4):
                            for c in range(4):
                                nc.tensor.matmul(
                                    sc[:, i, :],
                                    ctl_sb[:, c, i * 128:(i + 1) * 128],
                                    qabsT[:, c, :, lb],
                                    start=(c == 0), stop=False)
                            s0 = st * 512 + i * 128
                            nc.tensor.matmul(sc[:, i, :],
                                             ctr_sb[:, s0:s0 + 128],
                                             qpeT[:, :, lb],
                                             start=False, stop=True)
                        eT = etp.tile([128, 4, 16], BF16, tag="eT")
                        nc.scalar.activation(eT[:, :, :], sc[:, :, :], EXP,
                                             scale=SCALE)
                        natc = natp.tile([128, 4, KL], BF16, tag="nat")
                        for i in range(trf):
                            tr = pstr.tile([128, KL], BF16, tag="tr")
                            for c in range(4):
                                nc.tensor.transpose(
                                    tr[:, c * 128:(c + 1) * 128],
                                    ctl_sb[:, c, i * 128:(i + 1) * 128],
                                    identB[:, :])
                            _copy_eng(nc, st * 4 + i)(natc[:, i, :], tr[:, :])
                        if NSTR:
                            r0 = (lb * ST + st) * NSTR * 128
                            nc.sync.dma_start(
                                natc[:, trf:, :],
                                nat[r0:r0 + NSTR * 128, :]
                                .rearrange("(i p) c -> p i c", p=128))
                        if pend is not None:
                            emit_ctx(lb, pend[0], pend[1], pend[2], ctxa, sums)
                        pend = (st, eT, natc)
                    emit_ctx(lb, pend[0], pend[1], pend[2], ctxa, sums)

                    # ---- normalize: ctxTn[:, c, :, lb] = ctxa * (1/sums) ----
                    rec = smp.tile([1, 16], F32, tag="rec")
                    nc.vector.reciprocal(rec[:, :], sums[:1, :])
                    bcn = smp.tile([128, 16], F32, tag="bcnsb")
                    nc.gpsimd.partition_broadcast(bcn[:, :], rec[:1, :])
                    nb = psn.tile([128, 16], F32, tag="nrm", name=f"nrm{lb}")
                    for c in range(4):
                        nc.vector.tensor_tensor(ctxTn[:, c, :, lb],
                                                ctxa[:, c, :], bcn[:, :],
                                                MULT)

                    # ---- un-absorb this sequence (overlaps next seq) ----
                    for h in range(H):
                        for c in range(4):
                            nc.tensor.matmul(nb[:, h:h + 1],
                                             w_vc_sb[:, h, c, :],
                                             ctxTn[:, c, h, lb:lb + 1],
                                             start=(c == 0), stop=(c == 3))
                    nc.vector.tensor_copy(ov_sb[:, :, lb], nb[:, :])

            # ================= tail: unabsorb, AllGather, o_proj ==========
            with (
                tc.tile_pool(name="pst2", bufs=1, space="PSUM") as pst2,
            ):
                ps_rows = pst2.tile([BP, H * DV], BF16, name="ps_rows")
                for h in range(H):
                    nc.tensor.transpose(ps_rows[:BP, h * DV:(h + 1) * DV],
                                        ov_sb[:, h, :], identB[:, :])
                ov_rows = qsb.tile([BP, H * DV], BF16)
                nc.vector.tensor_copy(ov_rows[:, :H * DV // 2],
                                      ps_rows[:BP, :H * DV // 2])
                nc.scalar.copy(ov_rows[:, H * DV // 2:],
                               ps_rows[:BP, H * DV // 2:])
                ag_in = dramp.tile([BP, H * DV], BF16)
                ag_out = dramp.tile([B, H * DV], BF16)
                nc.sync.dma_start(ag_in[:, :], ov_rows[:, :])
                if fake_coll:
                    nc.sync.dma_start(ag_out[0:BP, :], ag_in[:, :])
                else:
                    nc.gpsimd.collective_compute(
                        "AllGather", BYPASS, replica_groups=rg,
                        ins=[ag_in.opt()], outs=[ag_out.opt()])
                ov32 = qsb.tile([B, H * DV], BF16)
                nc.sync.dma_start(ov32[:, :], ag_out[:, :])

            with (
                tc.tile_pool(name="pst3", bufs=1, space="PSUM") as pst3,
                tc.tile_pool(name="pst4", bufs=1, space="PSUM") as pst4,
            ):
                ps_tt = pst3.tile([128, 16, B], BF16, name="ps_tt")
                for kt in range(16):
                    nc.tensor.transpose(ps_tt[:, kt, :],
                                        ov32[:B, kt * 128:(kt + 1) * 128],
                                        identB[:B, :B])
                ovT_f = qsb.tile([128, 16, B], BF16)
                nc.vector.tensor_copy(ovT_f[:, :, :], ps_tt[:, :, :])

                out_ps = pst4.tile([128, 5, B], F32, name="out_ps")
                for t in range(5):
                    for kt in range(16):
                        nc.tensor.matmul(
                            out_ps[:, t, :],
                            w_o_sb[:, kt, t * 128:(t + 1) * 128],
                            ovT_f[:, kt, :],
                            start=(kt == 0), stop=(kt == 15))
                out_sb = qsb.tile([128, 5, B], F32)
                nc.scalar.copy(out_sb[:, :, :], out_ps[:, :, :])
                nc.sync.dma_start(
                    out[:, :],
                    out_sb[:, :, :].rearrange("p t b -> p (t b)"))

    nc.compile()
    return nc


# ----------------------------- host wrapper ------------------------------


def _prep_in_maps(inputs, S, n_cores, tp, trf=TRF):
    hidden = np.asarray(inputs["hidden_states"], np.float32)
    pos = np.asarray(inputs["positions"], np.int32)
    w_qkv_a = np.asarray(inputs["w_qkv_a"], np.float32)
    q_a_norm_w = np.asarray(inputs["q_a_norm_w"], np.float32)
    w_q_b = np.asarray(inputs["w_q_b"], np.float32)
    kv_a_norm_w = np.asarray(inputs["kv_a_norm_w"], np.float32)
    w_kc = np.asarray(inputs["w_kc"], np.float32)
    w_vc = np.asarray(inputs["w_vc"], np.float32)
    w_o = np.asarray(inputs["w_o"], np.float32)
    cache_l = np.asarray(inputs["kv_cache_latent"], np.float32)
    cache_r = np.asarray(inputs["kv_cache_rope"], np.float32)
    ST = S // 512
    NSTR = 4 - trf

    # current-token cache update (host)
    latent = hidden @ w_qkv_a[:, QL:QL + KL]
    k_pe = hidden @ w_qkv_a[:, QL + KL:]
    latent_n = _rmsnorm_np(latent, kv_a_norm_w)
    k_pe_r = _rope_np(k_pe.astype(np.float32), pos)
    cache_l = cache_l.copy()
    cache_r = cache_r.copy()
    cache_l[:, -1, :] = latent_n
    cache_r[:, -1, :] = k_pe_r
    cache_l_b = cache_l[:, :S, :].astype(NPBF)
    cache_r_b = cache_r[:, :S, :].astype(NPBF)

    hiddenT_b = np.ascontiguousarray(
        hidden.T.reshape(KTH, 128, B).transpose(1, 0, 2)).astype(NPBF)
    w_qb_eff = (q_a_norm_w[:, None] * w_q_b).astype(np.float32)
    RT = _rope_RT(pos)
    w_qa_q = w_qkv_a[:, :QL]
    w_kc_b = np.ascontiguousarray(
        w_kc.transpose(1, 0, 2)).astype(NPBF)            # [128, H, KL]
    w_vc_b = np.ascontiguousarray(
        w_vc.reshape(H, 4, 128, DV).transpose(2, 0, 1, 3)).astype(NPBF)

    in_maps = []
    for k in range(n_cores):
        b0 = k * BP
        cl = cache_l[b0:b0 + BP, :S, :]                  # fp32 view
        # transposed layout [b, st, p(c%128), ct, s]
        ctlT = (cl.transpose(0, 2, 1)
                .reshape(BP, 4, 128, ST, 512)
                .transpose(0, 3, 2, 1, 4))
        ctl_h = np.ascontiguousarray(ctlT).astype(NPBF).reshape(
            BP * ST * 128, 2048)
        ctr_h = np.ascontiguousarray(
            cache_r_b[b0:b0 + BP].transpose(0, 2, 1)).reshape(BP * 64, S)
        wqa_h = np.ascontiguousarray(
            w_qa_q[:, k * QS:(k + 1) * QS]
            .reshape(KTH, 128, QS).transpose(1, 0, 2)).astype(NPBF)
        wqb_pad = np.zeros((256, H * (DN + DR)), np.float32)
        wqb_pad[:QS] = w_qb_eff[k * QS:(k + 1) * QS]
        wqb_h = np.ascontiguousarray(
            wqb_pad.reshape(2, 128, -1).transpose(1, 0, 2)).astype(NPBF)
        wo_h = np.ascontiguousarray(
            w_o[:, k * HO:(k + 1) * HO]
            .reshape(16, 128, HO).transpose(1, 0, 2)).astype(NPBF)
        m = {
            "ctl": ctl_h,
            "ctr": np.ascontiguousarray(ctr_h),
            "hT": hiddenT_b.reshape(128, KTH * B),
            "w_qa": wqa_h.reshape(128, KTH * QS),
            "w_qb": wqb_h.reshape(128, -1),
            "w_kc": w_kc_b.reshape(128, H * KL),
            "w_vc": w_vc_b.reshape(128, H * KL),
            "w_o": wo_h.reshape(128, 16 * HO),
            "ropeRT": np.ascontiguousarray(RT[b0:b0 + BP]),
        }
        if NSTR:
            nat_h = (cache_l_b[b0:b0 + BP]
                     .reshape(BP, ST, 4, 128, KL)[:, :, trf:, :, :])
            m["nat"] = np.ascontiguousarray(nat_h).reshape(
                BP * ST * NSTR * 128, KL)
        in_maps.append(m)
    return in_maps


def _unshard(results, tp):
    cols = []
    for k in range(N_CORES):
        o = results[k]["out"].reshape(128, 5, B)
        cols.append(o.transpose(2, 1, 0).reshape(B, 5 * 128))
    return np.concatenate(cols, axis=1)


def run(inputs, S=4096, trace=False):
    key = (S, N_CORES, TP, TRF)
    if key not in _CACHE:
        _CACHE[key] = _build(S, N_CORES, TP, trf=TRF)
    nc = _CACHE[key]
    in_maps = _prep_in_maps(inputs, S, N_CORES, TP, trf=TRF)
    res = bass_utils.run_bass_kernel_spmd(
        nc, in_maps, core_ids=list(range(N_CORES)), trace=trace)
    return _unshard(res.results, TP), res


def kernel(**inputs) -> np.ndarray:
    out, _ = run(inputs)
    return out.astype(np.float32)
